# revision 10
# baseline (speedup 1.0000x reference)
"""Trainium2 Bass kernel: dense attention with key-padding mask (ColoAttention).

Math (per batch b, head h):
    scores = (Q @ K^T) / sqrt(D); masked keys -> -inf; softmax over keys;
    out = probs @ V; rows at masked query positions zeroed.

Implementation notes (v4):
  - K and V rows at masked key positions are zeroed on the host, so masked
    scores are exactly 0, exp(0) = 1, and the host subtracts the masked-key
    count from each row's sum of exponentials.  Masked V rows contribute 0.
  - The mask is a contiguous valid prefix; the host reads max_len from it and
    trims compute to NKC = ceil(max_len/128) key chunks and NKC*128 query
    columns (identical on every core, so the single SPMD program stays
    static).  Queries/keys beyond that are masked for every batch and are
    repadded with zeros on the host.
  - Scores are computed transposed (S^T[k, q] = K @ Q^T) so the exp output
    P^T (bf16) directly feeds O'^T = V^T @ P^T.
  - Each q-half is two 416-col windows inside a [128, 2, 512] PSUM tile: a
    single activation reads both windows as a two-run AP whose runs stay
    inside PSUM banks (a run crossing a bank boundary costs ~+180ns), and
    matmul outputs stay inside single banks.  Score tiles are triple-
    buffered (host-side sums freed the banks) so QK runs two steps ahead
    and the exp chain never waits on scores.
  - Row sums: DVE accumulates P^T chunks elementwise in bf16 (2x 16-bit
    mode); the per-half [128, QH] bf16 accumulator is DMAed out and the
    host does the final 128-way reduction during normalize/repad.
  - All matmuls are bf16 (f32r stationary weights load 4x slower).
  - Sharding: B*H = 64 (b,h) pairs; core c handles batch c//2, heads
    (c%2)*8 .. +8.  Pure SPMD, no collectives.
"""

import numpy as np
import ml_dtypes
from contextlib import ExitStack

import concourse.bass as bass
import concourse.mybir as mybir
import concourse.tile as tile
from concourse import bacc
from concourse.bass_utils import run_bass_kernel_spmd

B, S, H, D = 4, 2048, 16, 128
N_CORES = 8
CORES_PER_BATCH = N_CORES // B            # 2
HPC = H // CORES_PER_BATCH                # 8 (b,h) pairs per core
P = 128
SCALE = 1.0 / float(np.sqrt(np.float64(D)).astype(np.float32))


def build_program(n_pairs: int = HPC, nkc: int = 16) -> bacc.Bacc:
    """One core's program: n_pairs heads, nkc key chunks of 128, q range
    nkc*128 split into 2 halves of 2 windows each."""
    W = nkc * 32            # q window width
    QH = 2 * W              # q columns per half
    LQ = nkc * 128          # total q columns
    LK = nkc * 128          # total keys
    f32 = mybir.dt.float32
    bf16 = mybir.dt.bfloat16
    Exp = mybir.ActivationFunctionType.Exp
    Add = mybir.AluOpType.add

    nc = bacc.Bacc("TRN2", target_bir_lowering=False, debug=False)
    # q/k arrive pre-transposed from the host: [pair, D, S-trimmed]
    q_d = nc.dram_tensor("q", [n_pairs, P, LQ], bf16, kind="ExternalInput").ap()
    k_d = nc.dram_tensor("k", [n_pairs, P, LK], bf16, kind="ExternalInput").ap()
    v_d = nc.dram_tensor("v", [n_pairs, LK, P], bf16, kind="ExternalInput").ap()
    out_d = nc.dram_tensor("out", [n_pairs, P, LQ], f32, kind="ExternalOutput").ap()
    acc_d = nc.dram_tensor("acc_out", [n_pairs, 2, P, QH], bf16,
                           kind="ExternalOutput").ap()

    with tile.TileContext(nc) as tc:
        with ExitStack() as ctx:
            qtp = ctx.enter_context(tc.tile_pool(name="qtp", bufs=2))
            inp = ctx.enter_context(tc.tile_pool(name="inp", bufs=2))
            ptp = ctx.enter_context(tc.tile_pool(name="ptp", bufs=4))
            accp = ctx.enter_context(tc.tile_pool(name="accp", bufs=2))
            otp = ctx.enter_context(tc.tile_pool(name="otp", bufs=4))
            # PSUM (8 banks): scores 3 bufs x [128,2,512] f32 = 6 banks,
            # O' accum 2 x [128,W] = 2 banks.
            sps = ctx.enter_context(tc.tile_pool(name="sps", bufs=3, space="PSUM"))
            ops = ctx.enter_context(tc.tile_pool(name="ops", bufs=2, space="PSUM"))

            pair_tiles = {}

            def load_pair(p):
                qt = qtp.tile([P, LQ], bf16, tag="qt", name=f"qt_{p}")
                nc.sync.dma_start(qt[:], q_d[p])
                kt = qtp.tile([P, LK], bf16, tag="kt", name=f"kt_{p}")
                nc.sync.dma_start(kt[:], k_d[p])
                v = inp.tile([P, nkc, P], bf16, tag="v", name=f"v_{p}")
                nc.sync.dma_start(v[:], v_d[p].rearrange("(t r) d -> r t d", r=P))
                pair_tiles[p] = (qt, kt, v)

            gsteps = [(p, h, kc)
                      for p in range(n_pairs) for h in (0, 1)
                      for kc in range(nkc)]

            def emit_qk(p, h, kc, i):
                if p not in pair_tiles:
                    load_pair(p)
                qt, kt, v = pair_tiles[p]
                s = sps.tile([P, 2, 512], f32, tag="s", name=f"s_{i}")
                for w in (0, 1):
                    nc.tensor.matmul(
                        s[:, w, 0:W],
                        lhsT=kt[:, kc * P:(kc + 1) * P],
                        rhs=qt[:, h * QH + w * W: h * QH + (w + 1) * W],
                        start=True, stop=True)
                return s

            half_state = {}

            def emit_pv(p, h, kc, pt):
                # PV lags the exp by one step so the PE consumes the
                # previous step's probabilities and never idles (an idle
                # gap resets the PE to mid p-state, halving matmul rate).
                _, _, v = pair_tiles[p]
                o_ps, acc = half_state[(p, h)]
                for w in (0, 1):
                    nc.tensor.matmul(
                        o_ps[w][:],
                        lhsT=v[:, kc, :],
                        rhs=pt[:, w, :],
                        start=(kc == 0), stop=(kc == nkc - 1))
                if kc != nkc - 1:
                    return
                # ---- half tail: store O'^T and the bf16 sum partials (the
                # host does the 128-way reduction + normalize) ----
                for w in (0, 1):
                    o_sb = otp.tile([P, W], f32, tag="osb",
                                    name=f"osb_{p}_{h}_{w}")
                    nc.vector.tensor_copy(out=o_sb[:], in_=o_ps[w][:])
                    nc.sync.dma_start(
                        out_d[p][:, h * QH + w * W: h * QH + (w + 1) * W],
                        o_sb[:])
                nc.sync.dma_start(acc_d[p, h], acc[:])

            pend = {0: emit_qk(*gsteps[0], 0)}
            if len(gsteps) > 1:
                pend[1] = emit_qk(*gsteps[1], 1)
            prev = None
            for i, (p, h, kc) in enumerate(gsteps):
                if (p, h) not in half_state:
                    half_state[(p, h)] = (
                        [ops.tile([P, W], f32, tag="o", name=f"o_{p}_{h}_{w}")
                         for w in (0, 1)],
                        accp.tile([P, 2, W], bf16, tag="acc", name=f"acc_{p}_{h}"),
                    )
                _, acc = half_state[(p, h)]
                s = pend.pop(i)
                pt = ptp.tile([P, 2, W], bf16, tag="pt", name=f"pt_{i}")
                nc.scalar.activation(pt[:], s[:, :, 0:W], Exp, scale=SCALE)
                if i + 2 < len(gsteps):
                    pend[i + 2] = emit_qk(*gsteps[i + 2], i + 2)
                if prev is not None:
                    emit_pv(*prev)
                if kc == 0:
                    nc.vector.tensor_copy(out=acc[:], in_=pt[:])
                else:
                    nc.vector.tensor_tensor(out=acc[:], in0=acc[:], in1=pt[:],
                                            op=Add)
                prev = (p, h, kc, pt)
            emit_pv(*prev)

    nc.compile()
    return nc


_PROG_CACHE: dict = {}


def _get_program(nkc: int = 13) -> bacc.Bacc:
    if nkc not in _PROG_CACHE:
        _PROG_CACHE[nkc] = build_program(HPC, nkc)
    return _PROG_CACHE[nkc]


def nkc_for_mask(attn_mask) -> int:
    mf = np.asarray(attn_mask) > 0
    valid = np.nonzero(mf.any(axis=0))[0]
    maxlen = int(valid[-1]) + 1 if valid.size else 1
    return min(16, max(8, -(-maxlen // 128)))


def make_in_maps(query, key, value, attn_mask):
    nkc = nkc_for_mask(attn_mask)
    LQ = LK = nkc * 128
    bf = ml_dtypes.bfloat16
    qT = np.asarray(query, np.float32).transpose(0, 2, 3, 1)[:, :, :, :LQ]
    kT = np.asarray(key, np.float32).transpose(0, 2, 3, 1)       # [B, H, D, S]
    v = np.asarray(value, np.float32).transpose(0, 2, 1, 3)      # [B, H, S, D]
    mf = (np.asarray(attn_mask) > 0).astype(np.float32)          # [B, S]
    kTz = (kT * mf[:, None, None, :])[:, :, :, :LK]
    vz = (v * mf[:, None, :, None])[:, :, :LK, :].astype(bf)
    in_maps = []
    for c in range(N_CORES):
        b, h0 = c // CORES_PER_BATCH, (c % CORES_PER_BATCH) * HPC
        in_maps.append({
            "q": np.ascontiguousarray(qT[b, h0:h0 + HPC]).astype(bf),
            "k": np.ascontiguousarray(kTz[b, h0:h0 + HPC]).astype(bf),
            "v": np.ascontiguousarray(vz[b, h0:h0 + HPC]),
        })
    return in_maps, (mf, nkc)


def assemble_output(results, aux):
    mf, nkc = aux
    LQ = LK = nkc * 128
    # masked keys inside the computed window contribute exp(0)=1 to the sums
    mcount = (LK - mf[:, :LK].sum(axis=1)).astype(np.float32)    # [B]
    out = np.zeros((B, S, H * D), np.float32)
    for c in range(N_CORES):
        b, h0 = c // CORES_PER_BATCH, (c % CORES_PER_BATCH) * HPC
        oT = results[c]["out"]                                   # [HPC, D, LQ]
        # acc_out: [HPC, 2, 128, QH] bf16 partials -> reduce the 128 axis
        acc = results[c]["acc_out"].astype(np.float32)
        sums = acc.sum(axis=2).reshape(HPC, LQ) - mcount[b]      # [HPC, LQ]
        with np.errstate(divide="ignore", invalid="ignore"):
            scale = np.where(mf[b][None, :LQ] > 0, 1.0 / sums, 0.0)
        o = oT * scale[:, None, :]                               # [HPC, D, LQ]
        for i in range(HPC):
            out[b, :LQ, (h0 + i) * D:(h0 + i + 1) * D] = o[i].T
    for b in range(B):
        if mf[b].sum() == 0.0:                                   # degenerate
            out[b] = 0.0
    return out


def kernel(query, key, value, attn_mask):
    in_maps, aux = make_in_maps(query, key, value, attn_mask)
    nc = _get_program(aux[1])
    res = run_bass_kernel_spmd(nc, in_maps, list(range(N_CORES)))
    return assemble_output(res.results, aux)
